# revision 24
# baseline (speedup 1.0000x reference)
"""Head-sharded causal self-attention (QK-RMSNorm + RoPE + value-residual mix)
for 8 Trainium2 NeuronCores.

Sharding: 16 heads -> 2 heads per core (tensor parallel). Each core computes
its heads' QKV projections, attention, and a partial c_proj output
[D, T] (transposed, fp16); the host sums the 8 partials (the c_proj
all-reduce).

v2 design (vs the f32r baseline):
 - fp16 operands everywhere off-PSUM: QKV/score/AV/proj matmuls all run at
   1 cycle/row at any width, LDWEIGHTS halves, DVE element-wise ops hit the
   2x/4x 16-bit perf modes, and input+output DMA bytes halve.
 - The 288 softmax-denominator ones-matmuls (512 PE cycles each -- as costly
   as a full AV matmul) are gone: DVE accumulates the exp tiles per (j,head)
   chain (fp16, 4x mode), and a single [1,512] ones-matmul per (j,head)
   contracts the accumulated tile.
 - The QKV phase is woven into the attention kt-loops (wavefront): attention
   block j's inner loop emits the next t-tile group's QKV matmuls as filler,
   so the PE never idles during the ACT-exp-paced attention stretches and
   the phase barrier disappears.
 - PSUM budget is exactly 8 banks: 2 qkv halves + 2 o_ps + 4 shared-ring
   tiles (scores / transpose-packs / proj / sums share one tag).
 - exp is computed as exp(s - 1) (bias rides the activation) so e and the
   fp16 denominator accumulator stay comfortably inside fp16 range; the
   offset cancels in the softmax ratio.
"""

import numpy as np

import concourse.bacc as bacc
import concourse.mybir as mybir
import concourse.tile as tile
from concourse.bass_utils import run_bass_kernel_spmd

P = 128
T = 4096
D = 2048
HD = 128
NH = 16
HPC = 2            # heads per core
NCORES = 8
NT = T // P        # 32 t-tiles
KT = D // P        # 16 contraction tiles for the projections
NJ = 8             # q-blocks
QB = 512           # q-block width
EPS = 1.1920929e-07
EXP_BIAS = -1.0    # exp(s + EXP_BIAS); cancels in softmax, guards fp16 range

F32 = mybir.dt.float32
F16 = mybir.dt.float16


def _build():
    nc = bacc.Bacc("TRN2", target_bir_lowering=False, debug=False,
                   enable_asserts=False, num_devices=NCORES)

    # ---- DRAM parameters (host pre-tiled layouts, all fp16) ----
    xt = nc.dram_tensor("xt", [NT, P, KT, P], F16, kind="ExternalInput").ap()
    wall = nc.dram_tensor("wall", [P, KT, 6 * HD], F16, kind="ExternalInput").ap()
    wproj = nc.dram_tensor("wproj", [P, HPC, D], F16, kind="ExternalInput").ap()
    vilam = nc.dram_tensor("vilam", [NT, P, HPC * HD], F16, kind="ExternalInput").ap()
    cs = nc.dram_tensor("cs", [P, NT, HD], F16, kind="ExternalInput").ap()
    mask01 = nc.dram_tensor("mask01", [P, 896], F16, kind="ExternalInput").ap()
    ones = nc.dram_tensor("ones", [P, 1], F16, kind="ExternalInput").ap()
    identr = nc.dram_tensor("identr", [P, P], F16, kind="ExternalInput").ap()
    out = nc.dram_tensor("out", [D, T], F16, kind="ExternalOutput").ap()

    with tile.TileContext(nc) as tc:
        with tc.tile_pool(name="persist", bufs=1) as persist:
            # q^T/k^T fused [q0 q1 k0 k1] so one copy per t-tile evicts
            # all four transposes
            qk_T = persist.tile([P, 4, T], F16)
            v_sb = persist.tile([P, HPC, NT, HD], F16)  # v natural per head
            cs_sb = persist.tile([P, NT, HD], F16)
            wproj_sb = persist.tile([P, HPC, D], F16)
            mask_sb = persist.tile([P, 896], F16)
            ones_sb = persist.tile([P, 1], F16)
            ident = persist.tile([P, P], F16)
            eps_q = persist.tile([P, 1], F32)
            nbias = persist.tile([P, 1], F32)
            wall_sb = [persist.tile([P, 6 * HD], F16, name=f"wall_sb{kt}")
                       for kt in range(KT)]
            nc.gpsimd.memset(eps_q[:], float(P) * EPS)
            nc.gpsimd.memset(nbias[:], EXP_BIAS)
            # warm up the gpsimd partition_broadcast ucode path early: its
            # first invocation pays a ~7us pool reconfig we must keep off
            # the attention critical path
            warm = persist.tile([P, 8], F32)
            nc.gpsimd.partition_broadcast(warm[:], eps_q[0:1, 0:1]
                                          .broadcast_to([1, 8]))

            with tc.tile_pool(name="xp", bufs=8) as xp, \
                 tc.tile_pool(name="vip", bufs=8) as vip, \
                 tc.tile_pool(name="small", bufs=3) as small, \
                 tc.tile_pool(name="sq", bufs=2) as sqp, \
                 tc.tile_pool(name="ep", bufs=8) as ep, \
                 tc.tile_pool(name="accp", bufs=4) as accp, \
                 tc.tile_pool(name="osbp", bufs=4) as osbp, \
                 tc.tile_pool(name="bcp", bufs=2) as bcp, \
                 tc.tile_pool(name="pop", bufs=4) as pop, \
                 tc.tile_pool(name="qkvps", bufs=2, space="PSUM") as qkvps, \
                 tc.tile_pool(name="ops", bufs=2, space="PSUM") as ops_, \
                 tc.tile_pool(name="gen", bufs=4, space="PSUM") as gen:

                # ---------- startup DMAs (order matters on the sync queue)
                x_tiles = {}
                vi_tiles = {}

                def dma_x(tt, chunks=1):
                    x_t = xp.tile([P, KT, P], F16, tag="x", name=f"x_{tt}")
                    step = KT // chunks
                    for c in range(chunks):
                        nc.sync.dma_start(
                            out=x_t[:, c * step:(c + 1) * step, :],
                            in_=xt[tt, :, c * step:(c + 1) * step, :])
                    x_tiles[tt] = x_t

                def dma_vi(tt):
                    vi_t = vip.tile([P, HPC * HD], F16, tag="vi",
                                    name=f"vi_{tt}")
                    nc.sync.dma_start(out=vi_t[:], in_=vilam[tt])
                    vi_tiles[tt] = vi_t

                # startup: issue from four queues in parallel so the
                # first matmul's inputs land as early as possible
                nc.scalar.dma_start(out=wall_sb[0][:], in_=wall[:, 0, :])
                dma_x(0, chunks=8)
                for kt in range(1, KT):
                    nc.scalar.dma_start(out=wall_sb[kt][:],
                                        in_=wall[:, kt, :])
                dma_x(1, chunks=4)
                nc.gpsimd.dma_start(out=vi_tiles.setdefault(
                    0, vip.tile([P, HPC * HD], F16, tag="vi", name="vi_0"))[:],
                    in_=vilam[0])
                nc.gpsimd.dma_start(out=vi_tiles.setdefault(
                    1, vip.tile([P, HPC * HD], F16, tag="vi", name="vi_1"))[:],
                    in_=vilam[1])
                nc.gpsimd.dma_start(out=cs_sb[:], in_=cs[:])
                nc.gpsimd.dma_start(out=ident[:], in_=identr[:])
                dma_x(2)
                dma_x(3)
                nc.gpsimd.dma_start(out=vi_tiles.setdefault(
                    2, vip.tile([P, HPC * HD], F16, tag="vi", name="vi_2"))[:],
                    in_=vilam[2])
                nc.gpsimd.dma_start(out=vi_tiles.setdefault(
                    3, vip.tile([P, HPC * HD], F16, tag="vi", name="vi_3"))[:],
                    in_=vilam[3])
                nc.gpsimd.dma_start(out=mask_sb[:], in_=mask01[:])
                nc.gpsimd.dma_start(out=ones_sb[:], in_=ones[:])
                nc.gpsimd.dma_start(out=wproj_sb[:], in_=wproj[:])

                # ---------- QKV t-tile as a list of PE-sized quanta
                def build_qkv(tt):
                    """Return a list of closures; calling them in order emits
                    t-tile tt's QKV+norm+rope+transpose work in ~0.9us PE
                    quanta."""
                    state = {}

                    def q_prefetch():
                        if tt + 4 < NT and tt + 4 not in x_tiles:
                            dma_x(tt + 4)
                            dma_vi(tt + 4)

                    def q_mm0():
                        q_prefetch()
                        h0 = qkvps.tile([P, 512], F32, tag="qkv",
                                        name=f"h0_{tt}")
                        state["h0"] = h0
                        for kt in range(4):
                            nc.tensor.matmul(h0[:], x_tiles[tt][:, kt, :],
                                             wall_sb[kt][:, 0:512],
                                             start=(kt == 0), stop=False)

                    def mk_mm_h0(k0):
                        def q_mm():
                            h0 = state["h0"]
                            for kt in range(k0, k0 + 4):
                                nc.tensor.matmul(h0[:], x_tiles[tt][:, kt, :],
                                                 wall_sb[kt][:, 0:512],
                                                 start=False,
                                                 stop=(kt == KT - 1))
                        return q_mm

                    def q_mm_v0():
                        h1 = qkvps.tile([P, 256], F32, tag="qkv",
                                        name=f"h1_{tt}")
                        state["h1"] = h1
                        for kt in range(8):
                            nc.tensor.matmul(h1[:], x_tiles[tt][:, kt, :],
                                             wall_sb[kt][:, 512:768],
                                             start=(kt == 0), stop=False)

                    def q_mm_v1():
                        h1 = state["h1"]
                        for kt in range(8, KT):
                            nc.tensor.matmul(h1[:], x_tiles[tt][:, kt, :],
                                             wall_sb[kt][:, 512:768],
                                             start=False, stop=(kt == KT - 1))
                        # ---- drain: evict raw q,k then rms stats off SBUF
                        # (DVE may read only ONE psum operand per instruction)
                        h0 = state["h0"]
                        qkr = small.tile([P, 4, P], F16, tag="qkraw")
                        nc.vector.tensor_copy(qkr[:], h0[:])
                        state["qkr"] = qkr
                        sqo = sqp.tile([P, 4, P], F16, tag="sqo")
                        nc.vector.tensor_mul(sqo[:], qkr[:], qkr[:])
                        ssq = small.tile([P, 4], F32, tag="ssq")
                        nc.vector.tensor_reduce(
                            ssq[:], sqo[:], axis=mybir.AxisListType.X,
                            op=mybir.AluOpType.add)
                        # rsc = rsqrt(ssq) entirely on DVE so the ACT queue
                        # stays exp-only (a Sqrt would force an activation
                        # table reload on either side): seed (1/s)*(a+b*s)
                        # (~8% err over s in [32,288]), two Newton steps
                        sca = small.tile([P, 4], F32, tag="sca")
                        rsc = small.tile([P, 4], F32, tag="rsc")
                        nc.vector.reciprocal(rsc[:], ssq[:])
                        nc.vector.tensor_scalar(
                            out=sca[:], in0=ssq[:], scalar1=0.0442,
                            scalar2=4.6, op0=mybir.AluOpType.mult,
                            op1=mybir.AluOpType.add)
                        nc.vector.tensor_mul(rsc[:], rsc[:], sca[:])
                        for _ in range(2):
                            nc.vector.tensor_mul(sca[:], rsc[:], rsc[:])
                            nc.vector.tensor_mul(sca[:], sca[:], ssq[:])
                            nc.vector.tensor_scalar(
                                out=sca[:], in0=sca[:], scalar1=-0.5,
                                scalar2=1.5, op0=mybir.AluOpType.mult,
                                op1=mybir.AluOpType.add)
                            nc.vector.tensor_mul(rsc[:], rsc[:], sca[:])
                        state["rsc"] = rsc

                    def q_drain():
                        # scale q,k (rms + 1/sqrt(hd)) in fp16; v add; rope
                        h1, rsc = state["h1"], state["rsc"]
                        qkr = state["qkr"]
                        qk = small.tile([P, 4, P], F16, tag="qknat")
                        for i in range(4):      # q0 q1 k0 k1
                            nc.vector.tensor_scalar(
                                out=qk[:, i, :], in0=qkr[:, i, :],
                                scalar1=rsc[:, i:i + 1],
                                scalar2=(1.0 if i < 2 else float(np.sqrt(P))),
                                op0=mybir.AluOpType.mult,
                                op1=mybir.AluOpType.mult)
                        for h in range(HPC):
                            nc.vector.tensor_add(
                                v_sb[:, h, tt, :],
                                h1[:, h * P:(h + 1) * P],
                                vi_tiles[tt][:, h * P:(h + 1) * P])
                        # rope on all 4 tensors at once (fp16, 4x DVE mode)
                        rp = small.tile([P, 4, P], F16, tag="rope")
                        tmp = small.tile([P, 4, 64], F16, tag="ropetmp")
                        x1_ = qk[:, :, 0:64]
                        x2 = qk[:, :, 64:128]
                        cb = cs_sb[:, tt, None, 0:64].broadcast_to([P, 4, 64])
                        sb = cs_sb[:, tt, None, 64:128].broadcast_to([P, 4, 64])
                        nc.vector.tensor_mul(rp[:, :, 0:64], x1_, cb)
                        nc.vector.tensor_mul(tmp[:], x2, sb)
                        nc.vector.tensor_add(rp[:, :, 0:64],
                                             rp[:, :, 0:64], tmp[:])
                        nc.vector.tensor_mul(rp[:, :, 64:128], x2, cb)
                        nc.vector.tensor_mul(tmp[:], x1_, sb)
                        nc.vector.tensor_sub(rp[:, :, 64:128],
                                             rp[:, :, 64:128], tmp[:])
                        state["rp"] = rp

                    def q_transpose():
                        rp = state["rp"]
                        tp = gen.tile([P, 4, P], F16, tag="g",
                                      name=f"tp_{tt}")
                        for i in range(4):
                            nc.tensor.transpose(tp[:, i, :], rp[:, i, :],
                                                ident[:])
                        # one eviction for all four transposes
                        nc.scalar.copy(
                            qk_T[:, :, tt * P:(tt + 1) * P], tp[:])

                    return [q_mm0, mk_mm_h0(4), mk_mm_h0(8), mk_mm_h0(12),
                            q_mm_v0, q_mm_v1, q_drain, q_transpose]

                # ---------- proj of block j (from normalized o_sb pair)
                evict_flip = [0]

                def emit_proj(j, o_sb_pair):
                    # the final block's proj has no matmuls left to hide its
                    # evictions behind: spread its psum tiles over the now-
                    # idle qkv and o banks so evicts pipeline inside the
                    # matmul span instead of serializing after it
                    if j == NJ - 1:
                        pools = [(gen, "g"), (qkvps, "qkv"), (gen, "g"),
                                 (ops_, "ops")]
                    else:
                        pools = [(gen, "g")]
                    for dt_ in range(KT):
                        pool, ptag = pools[dt_ % len(pools)]
                        pp = pool.tile([P, QB], F32, tag=ptag,
                                       name=f"pp_{j}_{dt_}")
                        for h in range(HPC):
                            nc.tensor.matmul(
                                pp[:],
                                wproj_sb[:, h, dt_ * P:(dt_ + 1) * P],
                                o_sb_pair[h][:],
                                start=(h == 0), stop=(h == HPC - 1))
                        po = pop.tile([P, QB], F16, tag="po",
                                      name=f"po_{j}_{dt_}")
                        evict_flip[0] ^= 1
                        if evict_flip[0]:
                            nc.vector.tensor_copy(po[:], pp[:])
                        else:
                            nc.scalar.copy(po[:], pp[:])
                        nc.sync.dma_start(
                            out=out[dt_ * P:(dt_ + 1) * P,
                                    j * QB:(j + 1) * QB],
                            in_=po[:])

                # ---------- attention block j, weaving `quanta` as PE filler
                pending_proj = [None]

                def attention(j, quanta):
                    nkt = 4 * j + 4
                    qslice = qk_T[:, :, j * QB:(j + 1) * QB]
                    o_ps = [ops_.tile([P, QB], F32, tag="ops",
                                      name=f"ops_{j}_{h}")
                            for h in range(HPC)]
                    acc = [accp.tile([P, QB], F16, tag="acc",
                                     name=f"acc_{j}_{h}")
                           for h in range(HPC)]
                    exps = {}

                    def s_step(h, kt):
                        s_ps = gen.tile([P, QB], F32, tag="g",
                                        name=f"sps_{j}_{h}_{kt}")
                        nc.tensor.matmul(
                            s_ps[:],
                            qk_T[:, 2 + h, kt * P:(kt + 1) * P],
                            qslice[:, h, :],
                            start=True, stop=True)
                        e = ep.tile([P, QB], F16, tag="exp",
                                    name=f"exp_{j}_{h}_{kt}")
                        nc.scalar.activation(
                            e[:], s_ps[:], mybir.ActivationFunctionType.Exp,
                            bias=nbias[:], scale=1.0)
                        if kt >= 4 * j:   # diagonal band: zero masked entries
                            off = 384 - P * (kt - 4 * j)
                            nc.vector.tensor_mul(
                                e[:], e[:], mask_sb[:, off:off + QB])
                        # fold into the denominator accumulator (DVE fp16 4x)
                        if kt == 0:
                            pass          # wait for the pair
                        elif kt == 1:
                            nc.vector.tensor_add(acc[h][:],
                                                 exps[(h, 0)][:], e[:])
                        else:
                            nc.vector.tensor_add(acc[h][:], acc[h][:], e[:])
                        exps[(h, kt)] = e

                    def denom_chain(h):
                        # sum acc over partitions -> reciprocal -> broadcast
                        dsum = gen.tile([1, QB], F32, tag="g",
                                        name=f"dsum_{j}_{h}")
                        nc.tensor.matmul(dsum[:], ones_sb[:], acc[h][:],
                                         start=True, stop=True)
                        rsum = small.tile([1, QB], F32, tag="rsum")
                        rscr = small.tile([1, QB], F32, tag="rscr")
                        nc.vector.reciprocal_approx_accurate(
                            rsum[:], dsum[:], rscr[:])
                        bc = bcp.tile([P, QB], F32, tag="bc")
                        nc.gpsimd.partition_broadcast(bc[:], rsum[:])
                        return bc

                    early_bc = {}
                    # software pipeline: scores run two kt ahead
                    for h in range(HPC):
                        s_step(h, 0)
                    for h in range(HPC):
                        s_step(h, 1)
                    budget = 0.0
                    pace = len(quanta) / nkt if nkt else 0.0
                    qi = 0
                    for kt in range(nkt):
                        if kt == 1 and pending_proj[0] is not None:
                            emit_proj(*pending_proj[0])
                            pending_proj[0] = None
                        for h in range(HPC):
                            if kt + 2 < nkt:
                                s_step(h, kt + 2)
                            e = exps.pop((h, kt))
                            nc.tensor.matmul(o_ps[h][:], v_sb[:, h, kt, :],
                                             e[:],
                                             start=(kt == 0),
                                             stop=(kt == nkt - 1))
                        budget += pace
                        while qi < len(quanta) and budget >= 1.0:
                            quanta[qi]()
                            qi += 1
                            budget -= 1.0
                        if kt == nkt - 3:
                            # run the denominator chains while the last AVs
                            # stream so o-norm can fire immediately at the
                            # block end (shrinks every block boundary, and
                            # the final block's tail in particular)
                            for h in range(HPC):
                                early_bc[h] = denom_chain(h)
                    while qi < len(quanta):
                        quanta[qi]()
                        qi += 1

                    o_sb_pair = []
                    for h in range(HPC):
                        bc = early_bc.get(h)
                        if bc is None:
                            bc = denom_chain(h)
                        o_sb = osbp.tile([P, QB], F16, tag="osb")
                        nc.vector.tensor_mul(o_sb[:], o_ps[h][:], bc[:])
                        o_sb_pair.append(o_sb)

                    if j == NJ - 1:
                        emit_proj(j, o_sb_pair)
                    else:
                        pending_proj[0] = (j, o_sb_pair)

                # ---------- main wavefront
                for tt in range(4):
                    for q in build_qkv(tt):
                        q()
                for j in range(NJ):
                    if j < NJ - 1:
                        quanta = []
                        for tt in range(4 * j + 4, 4 * j + 8):
                            quanta.extend(build_qkv(tt))
                    else:
                        quanta = []
                    attention(j, quanta)

    nc.compile()
    return nc


_NC = None


def _get_nc():
    global _NC
    if _NC is None:
        _NC = _build()
    return _NC


def _host_inputs(x, vi, Wq, Wk, Wv, Wproj, lamb):
    """Build the per-core input maps (numpy, fp16 pre-tiled layouts)."""
    x = np.asarray(x, dtype=np.float32).reshape(T, D)
    vi = np.asarray(vi, dtype=np.float32).reshape(T, NH, HD)
    Wq = np.asarray(Wq, dtype=np.float32)
    Wk = np.asarray(Wk, dtype=np.float32)
    Wv = np.asarray(Wv, dtype=np.float32)
    Wproj = np.asarray(Wproj, dtype=np.float32)
    lam = float(np.asarray(lamb))

    # x^T tiled: xt[tt, p, kt, f] = x[tt*P+f, kt*P+p]
    xt = np.ascontiguousarray(
        x.reshape(NT, P, KT, P).transpose(0, 3, 2, 1)).astype(np.float16)

    # rope tables
    inv_freq = (1.0 / 10000.0) ** (np.arange(0, HD, 2, dtype=np.float32) / HD)
    tpos = np.arange(T, dtype=np.float32)
    freqs = np.outer(tpos, inv_freq).astype(np.float32)      # [T, 64]
    cs_full = np.concatenate([np.cos(freqs), np.sin(freqs)], axis=1)  # [T,128]
    cs_t = np.ascontiguousarray(
        cs_full.reshape(NT, P, HD).transpose(1, 0, 2)).astype(np.float16)

    # causal 0/1 mask bands: column c of slice offset (384-r) maps to
    # f-r = c-384; entry masked iff p > c-384
    m01 = (np.arange(P)[:, None] <= (np.arange(896)[None, :] - 384)
           ).astype(np.float16)
    ones = np.ones((P, 1), dtype=np.float16)

    in_maps = []
    for core in range(NCORES):
        r0 = core * HPC * HD
        wq_c = Wq[r0:r0 + HPC * HD]
        wk_c = Wk[r0:r0 + HPC * HD]
        wv_c = Wv[r0:r0 + HPC * HD] * (1.0 - lam)
        w_all = np.concatenate(
            [wq_c[0:HD], wq_c[HD:2 * HD],
             wk_c[0:HD], wk_c[HD:2 * HD],
             wv_c[0:HD], wv_c[HD:2 * HD]], axis=0)           # [768, D]
        # wall[p, kt, m] = w_all[m, kt*P+p]  (W_all^T tiled)
        wall_c = np.ascontiguousarray(
            w_all.reshape(6 * HD, KT, P).transpose(2, 1, 0)).astype(np.float16)
        # wproj[p, ct, m] = Wproj[m, r0 + ct*P + p]
        wp = Wproj[:, r0:r0 + HPC * HD]                       # [D, 256]
        wproj_c = np.ascontiguousarray(
            wp.reshape(D, HPC, P).transpose(2, 1, 0)).astype(np.float16)
        # vilam[tt, p, c] = lam * vi[tt*P+p, head, hd]
        vl = (lam * vi[:, HPC * core:HPC * (core + 1), :]).reshape(
            NT, P, HPC * HD)
        in_maps.append({
            "xt": xt,
            "wall": wall_c,
            "wproj": wproj_c,
            "vilam": np.ascontiguousarray(vl).astype(np.float16),
            "cs": cs_t,
            "mask01": m01,
            "ones": ones,
            "identr": np.eye(P, dtype=np.float16),
        })
    return in_maps


def kernel(x, vi, Wq, Wk, Wv, Wproj, lamb, _trace=False, _trace_kwargs=None):
    nc = _get_nc()
    in_maps = _host_inputs(x, vi, Wq, Wk, Wv, Wproj, lamb)
    res = run_bass_kernel_spmd(nc, in_maps, list(range(NCORES)),
                               trace=_trace, **(_trace_kwargs or {}))
    acc = np.zeros((D, T), dtype=np.float64)
    for core in range(NCORES):
        acc += res.results[core]["out"].astype(np.float64)
    y = acc.T.astype(np.float32).reshape(1, 1, T, D)
    if _trace:
        return y, res
    return y


# revision 26
# speedup vs baseline: 1.0674x; 1.0674x over previous
"""Head-sharded causal self-attention (QK-RMSNorm + RoPE + value-residual mix)
for 8 Trainium2 NeuronCores.

Sharding: 16 heads -> 2 heads per core (tensor parallel). Each core computes
its heads' QKV projections, attention, and a partial c_proj output
[D, T] (transposed, fp16); the host sums the 8 partials (the c_proj
all-reduce).

v2 design (vs the f32r baseline):
 - fp16 operands everywhere off-PSUM: QKV/score/AV/proj matmuls all run at
   1 cycle/row at any width, LDWEIGHTS halves, DVE element-wise ops hit the
   2x/4x 16-bit perf modes, and input+output DMA bytes halve.
 - The 288 softmax-denominator ones-matmuls (512 PE cycles each -- as costly
   as a full AV matmul) are gone: DVE accumulates the exp tiles per (j,head)
   chain (fp16, 4x mode), and a single [1,512] ones-matmul per (j,head)
   contracts the accumulated tile.
 - The QKV phase is woven into the attention kt-loops (wavefront): attention
   block j's inner loop emits the next t-tile group's QKV matmuls as filler,
   so the PE never idles during the ACT-exp-paced attention stretches and
   the phase barrier disappears.
 - PSUM budget is exactly 8 banks: 2 qkv halves + 2 o_ps + 4 shared-ring
   tiles (scores / transpose-packs / proj / sums share one tag).
 - exp is computed as exp(s - 1) (bias rides the activation) so e and the
   fp16 denominator accumulator stay comfortably inside fp16 range; the
   offset cancels in the softmax ratio.
"""

import numpy as np

import concourse.bacc as bacc
import concourse.mybir as mybir
import concourse.tile as tile
from concourse.bass_utils import run_bass_kernel_spmd

P = 128
T = 4096
D = 2048
HD = 128
NH = 16
HPC = 2            # heads per core
NCORES = 8
NT = T // P        # 32 t-tiles
KT = D // P        # 16 contraction tiles for the projections
NJ = 8             # q-blocks
QB = 512           # q-block width
EPS = 1.1920929e-07
EXP_BIAS = -1.0    # exp(s + EXP_BIAS); cancels in softmax, guards fp16 range

F32 = mybir.dt.float32
F16 = mybir.dt.float16


def _build():
    nc = bacc.Bacc("TRN2", target_bir_lowering=False, debug=False,
                   enable_asserts=False, num_devices=NCORES)

    # ---- DRAM parameters (host pre-tiled layouts, all fp16) ----
    xt = nc.dram_tensor("xt", [NT, P, KT, P], F16, kind="ExternalInput").ap()
    wall = nc.dram_tensor("wall", [P, KT, 6 * HD], F16, kind="ExternalInput").ap()
    wproj = nc.dram_tensor("wproj", [P, HPC, D], F16, kind="ExternalInput").ap()
    vilam = nc.dram_tensor("vilam", [NT, P, HPC * HD], F16, kind="ExternalInput").ap()
    cs = nc.dram_tensor("cs", [P, NT, HD], F16, kind="ExternalInput").ap()
    mask01 = nc.dram_tensor("mask01", [P, 896], F16, kind="ExternalInput").ap()
    ones = nc.dram_tensor("ones", [P, 1], F16, kind="ExternalInput").ap()
    identr = nc.dram_tensor("identr", [P, P], F16, kind="ExternalInput").ap()
    out = nc.dram_tensor("out", [D, T], F16, kind="ExternalOutput").ap()

    with tile.TileContext(nc) as tc:
        with tc.tile_pool(name="persist", bufs=1) as persist:
            # q^T/k^T fused [q0 q1 k0 k1] so one copy per t-tile evicts
            # all four transposes
            qk_T = persist.tile([P, 4, T], F16)
            v_sb = persist.tile([P, HPC, NT, HD], F16)  # v natural per head
            cs_sb = persist.tile([P, NT, HD], F16)
            wproj_sb = persist.tile([P, HPC, D], F16)
            mask_sb = persist.tile([P, 896], F16)
            ones_sb = persist.tile([P, 1], F16)
            ident = persist.tile([P, P], F16)
            eps_q = persist.tile([P, 1], F32)
            nbias = persist.tile([P, 1], F32)
            wall_sb = [persist.tile([P, 6 * HD], F16, name=f"wall_sb{kt}")
                       for kt in range(KT)]
            nc.gpsimd.memset(eps_q[:], float(P) * EPS)
            nc.gpsimd.memset(nbias[:], EXP_BIAS)
            # warm up the gpsimd partition_broadcast ucode path early: its
            # first invocation pays a ~7us pool reconfig we must keep off
            # the attention critical path
            warm = persist.tile([P, 8], F32)
            nc.gpsimd.partition_broadcast(warm[:], eps_q[0:1, 0:1]
                                          .broadcast_to([1, 8]))

            with tc.tile_pool(name="xp", bufs=8) as xp, \
                 tc.tile_pool(name="vip", bufs=8) as vip, \
                 tc.tile_pool(name="small", bufs=3) as small, \
                 tc.tile_pool(name="sq", bufs=2) as sqp, \
                 tc.tile_pool(name="ep", bufs=8) as ep, \
                 tc.tile_pool(name="accp", bufs=4) as accp, \
                 tc.tile_pool(name="osbp", bufs=4) as osbp, \
                 tc.tile_pool(name="bcp", bufs=2) as bcp, \
                 tc.tile_pool(name="pop", bufs=4) as pop, \
                 tc.tile_pool(name="qkvps", bufs=2, space="PSUM") as qkvps, \
                 tc.tile_pool(name="ops", bufs=2, space="PSUM") as ops_, \
                 tc.tile_pool(name="gen", bufs=4, space="PSUM") as gen:

                # ---------- startup DMAs (order matters on the sync queue)
                x_tiles = {}
                vi_tiles = {}

                def dma_x(tt, chunks=1):
                    x_t = xp.tile([P, KT, P], F16, tag="x", name=f"x_{tt}")
                    step = KT // chunks
                    for c in range(chunks):
                        nc.sync.dma_start(
                            out=x_t[:, c * step:(c + 1) * step, :],
                            in_=xt[tt, :, c * step:(c + 1) * step, :])
                    x_tiles[tt] = x_t

                def dma_vi(tt):
                    vi_t = vip.tile([P, HPC * HD], F16, tag="vi",
                                    name=f"vi_{tt}")
                    nc.sync.dma_start(out=vi_t[:], in_=vilam[tt])
                    vi_tiles[tt] = vi_t

                nc.sync.dma_start(out=wall_sb[0][:], in_=wall[:, 0, :])
                dma_x(0, chunks=8)
                for kt in range(1, 4):
                    nc.sync.dma_start(out=wall_sb[kt][:], in_=wall[:, kt, :])
                dma_x(1, chunks=4)
                dma_vi(0)
                dma_vi(1)
                for kt in range(4, KT):
                    nc.sync.dma_start(out=wall_sb[kt][:], in_=wall[:, kt, :])
                nc.sync.dma_start(out=cs_sb[:], in_=cs[:])
                nc.sync.dma_start(out=ident[:], in_=identr[:])
                dma_x(2)
                dma_x(3)
                dma_vi(2)
                dma_vi(3)
                nc.sync.dma_start(out=mask_sb[:], in_=mask01[:])
                nc.sync.dma_start(out=ones_sb[:], in_=ones[:])
                nc.sync.dma_start(out=wproj_sb[:], in_=wproj[:])

                # ---------- QKV t-tile as a list of PE-sized quanta
                def build_qkv(tt):
                    """Return a list of closures; calling them in order emits
                    t-tile tt's QKV+norm+rope+transpose work in ~0.9us PE
                    quanta."""
                    state = {}

                    def q_prefetch():
                        if tt + 4 < NT and tt + 4 not in x_tiles:
                            dma_x(tt + 4)
                            dma_vi(tt + 4)

                    def q_mm0():
                        q_prefetch()
                        h0 = qkvps.tile([P, 512], F32, tag="qkv",
                                        name=f"h0_{tt}")
                        state["h0"] = h0
                        for kt in range(4):
                            nc.tensor.matmul(h0[:], x_tiles[tt][:, kt, :],
                                             wall_sb[kt][:, 0:512],
                                             start=(kt == 0), stop=False)

                    def mk_mm_h0(k0):
                        def q_mm():
                            h0 = state["h0"]
                            for kt in range(k0, k0 + 4):
                                nc.tensor.matmul(h0[:], x_tiles[tt][:, kt, :],
                                                 wall_sb[kt][:, 0:512],
                                                 start=False,
                                                 stop=(kt == KT - 1))
                        return q_mm

                    def q_mm_v0():
                        h1 = qkvps.tile([P, 256], F32, tag="qkv",
                                        name=f"h1_{tt}")
                        state["h1"] = h1
                        for kt in range(8):
                            nc.tensor.matmul(h1[:], x_tiles[tt][:, kt, :],
                                             wall_sb[kt][:, 512:768],
                                             start=(kt == 0), stop=False)

                    def q_mm_v1():
                        h1 = state["h1"]
                        for kt in range(8, KT):
                            nc.tensor.matmul(h1[:], x_tiles[tt][:, kt, :],
                                             wall_sb[kt][:, 512:768],
                                             start=False, stop=(kt == KT - 1))
                        # ---- drain: evict raw q,k then rms stats off SBUF
                        # (DVE may read only ONE psum operand per instruction)
                        h0 = state["h0"]
                        qkr = small.tile([P, 4, P], F16, tag="qkraw")
                        nc.vector.tensor_copy(qkr[:], h0[:])
                        state["qkr"] = qkr
                        sqo = sqp.tile([P, 4, P], F16, tag="sqo")
                        nc.vector.tensor_mul(sqo[:], qkr[:], qkr[:])
                        ssq = small.tile([P, 4], F32, tag="ssq")
                        nc.vector.tensor_reduce(
                            ssq[:], sqo[:], axis=mybir.AxisListType.X,
                            op=mybir.AluOpType.add)
                        # rsc = rsqrt(ssq) entirely on DVE so the ACT queue
                        # stays exp-only (a Sqrt would force an activation
                        # table reload on either side): seed (1/s)*(a+b*s)
                        # (~8% err over s in [32,288]), two Newton steps
                        sca = small.tile([P, 4], F32, tag="sca")
                        rsc = small.tile([P, 4], F32, tag="rsc")
                        nc.vector.reciprocal(rsc[:], ssq[:])
                        nc.vector.tensor_scalar(
                            out=sca[:], in0=ssq[:], scalar1=0.0442,
                            scalar2=4.6, op0=mybir.AluOpType.mult,
                            op1=mybir.AluOpType.add)
                        nc.vector.tensor_mul(rsc[:], rsc[:], sca[:])
                        for _ in range(2):
                            nc.vector.tensor_mul(sca[:], rsc[:], rsc[:])
                            nc.vector.tensor_mul(sca[:], sca[:], ssq[:])
                            nc.vector.tensor_scalar(
                                out=sca[:], in0=sca[:], scalar1=-0.5,
                                scalar2=1.5, op0=mybir.AluOpType.mult,
                                op1=mybir.AluOpType.add)
                            nc.vector.tensor_mul(rsc[:], rsc[:], sca[:])
                        state["rsc"] = rsc

                    def q_drain():
                        # scale q,k (rms + 1/sqrt(hd)) in fp16; v add; rope
                        h1, rsc = state["h1"], state["rsc"]
                        qkr = state["qkr"]
                        qk = small.tile([P, 4, P], F16, tag="qknat")
                        for i in range(4):      # q0 q1 k0 k1
                            nc.vector.tensor_scalar(
                                out=qk[:, i, :], in0=qkr[:, i, :],
                                scalar1=rsc[:, i:i + 1],
                                scalar2=(1.0 if i < 2 else float(np.sqrt(P))),
                                op0=mybir.AluOpType.mult,
                                op1=mybir.AluOpType.mult)
                        for h in range(HPC):
                            nc.vector.tensor_add(
                                v_sb[:, h, tt, :],
                                h1[:, h * P:(h + 1) * P],
                                vi_tiles[tt][:, h * P:(h + 1) * P])
                        # rope on all 4 tensors at once (fp16, 4x DVE mode)
                        rp = small.tile([P, 4, P], F16, tag="rope")
                        tmp = small.tile([P, 4, 64], F16, tag="ropetmp")
                        x1_ = qk[:, :, 0:64]
                        x2 = qk[:, :, 64:128]
                        cb = cs_sb[:, tt, None, 0:64].broadcast_to([P, 4, 64])
                        sb = cs_sb[:, tt, None, 64:128].broadcast_to([P, 4, 64])
                        nc.vector.tensor_mul(rp[:, :, 0:64], x1_, cb)
                        nc.vector.tensor_mul(tmp[:], x2, sb)
                        nc.vector.tensor_add(rp[:, :, 0:64],
                                             rp[:, :, 0:64], tmp[:])
                        nc.vector.tensor_mul(rp[:, :, 64:128], x2, cb)
                        nc.vector.tensor_mul(tmp[:], x1_, sb)
                        nc.vector.tensor_sub(rp[:, :, 64:128],
                                             rp[:, :, 64:128], tmp[:])
                        state["rp"] = rp

                    def q_transpose():
                        rp = state["rp"]
                        tp = gen.tile([P, 4, P], F16, tag="g",
                                      name=f"tp_{tt}")
                        for i in range(4):
                            nc.tensor.transpose(tp[:, i, :], rp[:, i, :],
                                                ident[:])
                        # one eviction for all four transposes
                        nc.scalar.copy(
                            qk_T[:, :, tt * P:(tt + 1) * P], tp[:])

                    return [q_mm0, mk_mm_h0(4), mk_mm_h0(8), mk_mm_h0(12),
                            q_mm_v0, q_mm_v1, q_drain, q_transpose]

                # ---------- proj of block j (from normalized o_sb pair)
                evict_flip = [0]

                def emit_proj(j, o_sb_pair):
                    # the final block's proj has no matmuls left to hide its
                    # evictions behind: spread its psum tiles over the now-
                    # idle qkv and o banks so evicts pipeline inside the
                    # matmul span instead of serializing after it
                    if j == NJ - 1:
                        pools = [(gen, "g"), (qkvps, "qkv"), (gen, "g"),
                                 (ops_, "ops")]
                    else:
                        pools = [(gen, "g")]
                    for dt_ in range(KT):
                        pool, ptag = pools[dt_ % len(pools)]
                        pp = pool.tile([P, QB], F32, tag=ptag,
                                       name=f"pp_{j}_{dt_}")
                        for h in range(HPC):
                            nc.tensor.matmul(
                                pp[:],
                                wproj_sb[:, h, dt_ * P:(dt_ + 1) * P],
                                o_sb_pair[h][:],
                                start=(h == 0), stop=(h == HPC - 1))
                        po = pop.tile([P, QB], F16, tag="po",
                                      name=f"po_{j}_{dt_}")
                        evict_flip[0] ^= 1
                        if evict_flip[0]:
                            nc.vector.tensor_copy(po[:], pp[:])
                        else:
                            nc.scalar.copy(po[:], pp[:])
                        nc.sync.dma_start(
                            out=out[dt_ * P:(dt_ + 1) * P,
                                    j * QB:(j + 1) * QB],
                            in_=po[:])

                # ---------- attention block j, weaving `quanta` as PE filler
                pending_proj = [None]

                def attention(j, quanta):
                    nkt = 4 * j + 4
                    qslice = qk_T[:, :, j * QB:(j + 1) * QB]
                    o_ps = [ops_.tile([P, QB], F32, tag="ops",
                                      name=f"ops_{j}_{h}")
                            for h in range(HPC)]
                    acc = [accp.tile([P, QB], F16, tag="acc",
                                     name=f"acc_{j}_{h}")
                           for h in range(HPC)]
                    exps = {}

                    def s_step(h, kt):
                        s_ps = gen.tile([P, QB], F32, tag="g",
                                        name=f"sps_{j}_{h}_{kt}")
                        nc.tensor.matmul(
                            s_ps[:],
                            qk_T[:, 2 + h, kt * P:(kt + 1) * P],
                            qslice[:, h, :],
                            start=True, stop=True)
                        e = ep.tile([P, QB], F16, tag="exp",
                                    name=f"exp_{j}_{h}_{kt}")
                        nc.scalar.activation(
                            e[:], s_ps[:], mybir.ActivationFunctionType.Exp,
                            bias=nbias[:], scale=1.0)
                        if kt >= 4 * j:   # diagonal band: zero masked entries
                            off = 384 - P * (kt - 4 * j)
                            nc.vector.tensor_mul(
                                e[:], e[:], mask_sb[:, off:off + QB])
                        # fold into the denominator accumulator (DVE fp16 4x)
                        if kt == 0:
                            pass          # wait for the pair
                        elif kt == 1:
                            nc.vector.tensor_add(acc[h][:],
                                                 exps[(h, 0)][:], e[:])
                        else:
                            nc.vector.tensor_add(acc[h][:], acc[h][:], e[:])
                        exps[(h, kt)] = e

                    def denom_chain(h):
                        # sum acc over partitions -> reciprocal -> broadcast
                        dsum = gen.tile([1, QB], F32, tag="g",
                                        name=f"dsum_{j}_{h}")
                        nc.tensor.matmul(dsum[:], ones_sb[:], acc[h][:],
                                         start=True, stop=True)
                        rsum = small.tile([1, QB], F32, tag="rsum")
                        rscr = small.tile([1, QB], F32, tag="rscr")
                        nc.vector.reciprocal_approx_accurate(
                            rsum[:], dsum[:], rscr[:])
                        bc = bcp.tile([P, QB], F32, tag="bc")
                        nc.gpsimd.partition_broadcast(bc[:], rsum[:])
                        return bc

                    early_bc = {}
                    # software pipeline: scores run two kt ahead
                    for h in range(HPC):
                        s_step(h, 0)
                    for h in range(HPC):
                        s_step(h, 1)
                    budget = 0.0
                    pace = len(quanta) / nkt if nkt else 0.0
                    qi = 0
                    for kt in range(nkt):
                        if kt == 1 and pending_proj[0] is not None:
                            emit_proj(*pending_proj[0])
                            pending_proj[0] = None
                        for h in range(HPC):
                            if kt + 2 < nkt:
                                s_step(h, kt + 2)
                            e = exps.pop((h, kt))
                            nc.tensor.matmul(o_ps[h][:], v_sb[:, h, kt, :],
                                             e[:],
                                             start=(kt == 0),
                                             stop=(kt == nkt - 1))
                        budget += pace
                        while qi < len(quanta) and budget >= 1.0:
                            quanta[qi]()
                            qi += 1
                            budget -= 1.0
                        if j == NJ - 1 and kt == nkt - 3:
                            # final block has no later proj to hide behind:
                            # run the denominator chains while the last AVs
                            # stream so o-norm fires immediately after
                            for h in range(HPC):
                                early_bc[h] = denom_chain(h)
                    while qi < len(quanta):
                        quanta[qi]()
                        qi += 1

                    o_sb_pair = []
                    for h in range(HPC):
                        bc = early_bc.get(h)
                        if bc is None:
                            bc = denom_chain(h)
                        o_sb = osbp.tile([P, QB], F16, tag="osb")
                        nc.vector.tensor_mul(o_sb[:], o_ps[h][:], bc[:])
                        o_sb_pair.append(o_sb)

                    if j == NJ - 1:
                        emit_proj(j, o_sb_pair)
                    else:
                        pending_proj[0] = (j, o_sb_pair)

                # ---------- main wavefront
                for tt in range(4):
                    for q in build_qkv(tt):
                        q()
                for j in range(NJ):
                    if j < NJ - 1:
                        quanta = []
                        for tt in range(4 * j + 4, 4 * j + 8):
                            quanta.extend(build_qkv(tt))
                    else:
                        quanta = []
                    attention(j, quanta)

    nc.compile()
    return nc


_NC = None


def _get_nc():
    global _NC
    if _NC is None:
        _NC = _build()
    return _NC


def _host_inputs(x, vi, Wq, Wk, Wv, Wproj, lamb):
    """Build the per-core input maps (numpy, fp16 pre-tiled layouts)."""
    x = np.asarray(x, dtype=np.float32).reshape(T, D)
    vi = np.asarray(vi, dtype=np.float32).reshape(T, NH, HD)
    Wq = np.asarray(Wq, dtype=np.float32)
    Wk = np.asarray(Wk, dtype=np.float32)
    Wv = np.asarray(Wv, dtype=np.float32)
    Wproj = np.asarray(Wproj, dtype=np.float32)
    lam = float(np.asarray(lamb))

    # x^T tiled: xt[tt, p, kt, f] = x[tt*P+f, kt*P+p]
    xt = np.ascontiguousarray(
        x.reshape(NT, P, KT, P).transpose(0, 3, 2, 1)).astype(np.float16)

    # rope tables
    inv_freq = (1.0 / 10000.0) ** (np.arange(0, HD, 2, dtype=np.float32) / HD)
    tpos = np.arange(T, dtype=np.float32)
    freqs = np.outer(tpos, inv_freq).astype(np.float32)      # [T, 64]
    cs_full = np.concatenate([np.cos(freqs), np.sin(freqs)], axis=1)  # [T,128]
    cs_t = np.ascontiguousarray(
        cs_full.reshape(NT, P, HD).transpose(1, 0, 2)).astype(np.float16)

    # causal 0/1 mask bands: column c of slice offset (384-r) maps to
    # f-r = c-384; entry masked iff p > c-384
    m01 = (np.arange(P)[:, None] <= (np.arange(896)[None, :] - 384)
           ).astype(np.float16)
    ones = np.ones((P, 1), dtype=np.float16)

    in_maps = []
    for core in range(NCORES):
        r0 = core * HPC * HD
        wq_c = Wq[r0:r0 + HPC * HD]
        wk_c = Wk[r0:r0 + HPC * HD]
        wv_c = Wv[r0:r0 + HPC * HD] * (1.0 - lam)
        w_all = np.concatenate(
            [wq_c[0:HD], wq_c[HD:2 * HD],
             wk_c[0:HD], wk_c[HD:2 * HD],
             wv_c[0:HD], wv_c[HD:2 * HD]], axis=0)           # [768, D]
        # wall[p, kt, m] = w_all[m, kt*P+p]  (W_all^T tiled)
        wall_c = np.ascontiguousarray(
            w_all.reshape(6 * HD, KT, P).transpose(2, 1, 0)).astype(np.float16)
        # wproj[p, ct, m] = Wproj[m, r0 + ct*P + p]
        wp = Wproj[:, r0:r0 + HPC * HD]                       # [D, 256]
        wproj_c = np.ascontiguousarray(
            wp.reshape(D, HPC, P).transpose(2, 1, 0)).astype(np.float16)
        # vilam[tt, p, c] = lam * vi[tt*P+p, head, hd]
        vl = (lam * vi[:, HPC * core:HPC * (core + 1), :]).reshape(
            NT, P, HPC * HD)
        in_maps.append({
            "xt": xt,
            "wall": wall_c,
            "wproj": wproj_c,
            "vilam": np.ascontiguousarray(vl).astype(np.float16),
            "cs": cs_t,
            "mask01": m01,
            "ones": ones,
            "identr": np.eye(P, dtype=np.float16),
        })
    return in_maps


def kernel(x, vi, Wq, Wk, Wv, Wproj, lamb, _trace=False, _trace_kwargs=None):
    nc = _get_nc()
    in_maps = _host_inputs(x, vi, Wq, Wk, Wv, Wproj, lamb)
    res = run_bass_kernel_spmd(nc, in_maps, list(range(NCORES)),
                               trace=_trace, **(_trace_kwargs or {}))
    acc = np.zeros((D, T), dtype=np.float64)
    for core in range(NCORES):
        acc += res.results[core]["out"].astype(np.float64)
    y = acc.T.astype(np.float32).reshape(1, 1, T, D)
    if _trace:
        return y, res
    return y
